# revision 1
# baseline (speedup 1.0000x reference)
"""Trainium2 Bass kernel: batched locally-weighted ridge regression.

Per test point t: K[t,n] = exp(-|xte_t - xtr_n|^2 / (2 ls^2));
  A_t = Xtild^T diag(K[t]) Xtild + REG*I ; b_t = Xtild^T (K[t] * Y)
  ypred_t = xtild_t . A_t^{-1} b_t
Sharding: data-parallel over the 4096 test points -> 8 cores x 512.

On-device math uses a scaled kernel K'[t,n] = exp((S[n,t] - sn[n]/2) * c2)
(c2 = 1/ls^2), i.e. the exp(-st*c2/2) per-test factor is dropped; this
rescales A_t and b_t identically, so beta is preserved by using a
per-test ridge REG_t = REG * exp(st*c2/2).

Pipeline per core:
  PE : 16 transposes, gram S = Xtr @ Xte^T, big matmul K'^T-chunks @ [Z | Xtild*Y]
  ACT: fused exp(S*c2 - sn*c2/2), PSUM evacuations, copies
  DVE: Z build (outer products via stride-0 APs), batched Gaussian
       elimination + back-substitution (batch on partitions, 4 blocks of
       128 systems in the free dim), predictions.
"""

import numpy as np

import concourse.bacc as bacc
import concourse.bass as bass
import concourse.mybir as mybir
from concourse.bass import ds, ts
from concourse.bass_utils import run_bass_kernel_spmd
from concourse.tile import TileContext

F32 = mybir.dt.float32
P = 128
N_TRAIN = 2048
D = 31
DP = 32          # 1 + D
W = 33           # DP + rhs column
N_TEST = 4096
NCORES = 8
TS = N_TEST // NCORES   # 512 test points per core
NT = TS // P            # 4 t-tiles
NK = N_TRAIN // P       # 16 train chunks
REG = 1e-6
LNREG = float(np.log(REG))
F32R = mybir.dt.float32r
MM_FP32R = False     # f32r measured 1.1e-2 rel err on HW (vs 3.6e-5 fp32)


def _build_nc(c2: float):
    """Build the single-core Bass program (SPMD across 8 cores)."""
    nc = bacc.Bacc(trn_type="TRN2")

    xtr_d = nc.dram_tensor("xtrain", [N_TRAIN, D], F32, kind="ExternalInput")
    ytr_d = nc.dram_tensor("ytrain", [N_TRAIN, 1], F32, kind="ExternalInput")
    xte_d = nc.dram_tensor("xtest", [TS, D], F32, kind="ExternalInput")
    # transposed features packed for 4-way row-group gram matmuls:
    # [32g+d, cc*128+p] = Xtrain[(4*cc+g)*128+p, d]; cols 512: = XtestT x4
    xT_d = nc.dram_tensor("xT", [P, 4 * P + TS], F32, kind="ExternalInput")
    out_d = nc.dram_tensor("ypred", [TS, 1], F32, kind="ExternalOutput")

    c2h = 0.5 * c2

    with TileContext(nc) as tc:
        with (
            tc.tile_pool(name="const", bufs=1) as const,
            tc.tile_pool(name="sb", bufs=1) as sb,
            tc.tile_pool(name="pgram", bufs=4, space="PSUM") as pgram,
            tc.tile_pool(name="pxwx", bufs=4, space="PSUM") as pxwx,
        ):
            # ---- load inputs ----
            xtr = sb.tile([P, NK, D], F32)       # natural layout chunks
            nc.sync.dma_start(
                xtr, xtr_d.rearrange("(c p) d -> p c d", p=P)
            )
            ytr = sb.tile([P, NK], F32)
            nc.sync.dma_start(
                ytr, ytr_d.rearrange("(c p) one -> p (c one)", p=P)
            )
            xte = sb.tile([P, NT, D], F32)
            nc.sync.dma_start(
                xte, xte_d.rearrange("(t p) d -> p t d", p=P)
            )

            # ---- transposed inputs, row-group packed [128, 4*128+512] ----
            xT = sb.tile([P, 4 * P + TS], F32)
            nc.sync.dma_start(xT, xT_d[:, :])

            # ---- Xtild chunks [128, NK, 32] (ones column + Xtrain) ----
            xt = sb.tile([P, NK, DP], F32)
            nc.vector.memset(xt[:, :, 0:1], 1.0)
            nc.scalar.copy(xt[:, :, 1:DP], xtr)

            # ---- Z = [xtild_d * xtild_e (1024) | xtild * y (32)] ----
            MMDT = F32R if MM_FP32R else F32
            H = 16
            NZ = DP * H + H * H + DP             # 512 + 256 + 32 = 800
            zz = sb.tile([P, NK, NZ], MMDT)
            nc.vector.tensor_mul(
                zz[:, :, 0:DP * H].rearrange("p k (d e) -> p k d e", d=DP),
                xt[:, :, :, None].broadcast_to([P, NK, DP, H]),
                xt[:, :, None, H:DP].broadcast_to([P, NK, DP, H]),
            )
            nc.vector.tensor_mul(
                zz[:, :, DP * H:DP * H + H * H].rearrange(
                    "p k (d e) -> p k d e", d=H),
                xt[:, :, 0:H, None].broadcast_to([P, NK, H, H]),
                xt[:, :, None, 0:H].broadcast_to([P, NK, H, H]),
            )
            nc.vector.tensor_mul(
                zz[:, :, DP * H + H * H:], xt,
                ytr[:, :, None].broadcast_to([P, NK, DP]),
            )

            # ---- squared norms and per-partition exp biases ----
            sqtr = sb.tile([P, NK, D], F32)
            sn = sb.tile([P, NK], F32)
            nc.vector.tensor_mul(sqtr, xtr, xtr)
            nc.vector.tensor_reduce(
                sn, sqtr, mybir.AxisListType.X, mybir.AluOpType.add,
            )
            sqte = sb.tile([P, NT, D], F32)
            st = sb.tile([P, NT], F32)
            nc.vector.tensor_mul(sqte, xte, xte)
            nc.vector.tensor_reduce(
                st, sqte, mybir.AxisListType.X, mybir.AluOpType.add,
            )
            bias_n = sb.tile([P, NK], F32)       # -sn * c2/2
            nc.vector.tensor_scalar_mul(bias_n, sn, -c2h)
            # per-test ridge REG_t = exp(st*c2/2 + ln(REG)), [128, NT]
            lnreg_t = const.tile([P, 1], F32)
            nc.vector.memset(lnreg_t, LNREG)
            regt = sb.tile([P, NT], F32)
            nc.scalar.activation(
                regt, st, mybir.ActivationFunctionType.Exp,
                bias=lnreg_t[:, :], scale=c2h,
            )

            # ---- gram + K' = exp(S*c2 - sn*c2/2), layout [n_chunk, t] ----
            kp = sb.tile([P, NK, TS], MMDT)
            for cc in range(NK // 4):
                for g in range(4):
                    c = 4 * cc + g
                    sg = pgram.tile([P, TS], F32, tag="sg")
                    nc.tensor.matmul(
                        sg,
                        xT[32 * g:32 * g + D, ts(cc, P)],
                        xT[32 * g:32 * g + D, 4 * P:],
                        start=True, stop=True,
                        tile_position=(32 * g, 0),
                    )
                    nc.scalar.activation(
                        kp[:, c, :], sg, mybir.ActivationFunctionType.Exp,
                        bias=bias_n[:, ds(c, 1)], scale=c2,
                    )

            # ---- XWX | XWy: [512, 1056] per core via K'-chunks @ ZZ ----
            # ga holds [A | b] per system: [128 part(t), NT blocks, 32 rows, 33 cols]
            ga = sb.tile([P, NT, DP, W], F32)
            CHUNKS = [(0, 512), (512, 800)]
            for t in range(NT):
                for (c0, c1) in CHUNKS:
                    w = c1 - c0
                    px = pxwx.tile([P, 512], F32, tag="px")
                    for c in range(NK):
                        nc.tensor.matmul(
                            px[:, :w],
                            kp[:, c, ts(t, P)],
                            zz[:, c, c0:c1],
                            start=(c == 0), stop=(c == NK - 1),
                        )
                    if c0 == 0:
                        # cols e=16..31, all rows d
                        nc.scalar.copy(
                            ga[:, t, :, H:DP],
                            px[:, :w].rearrange("p (r c) -> p r c", r=DP),
                        )
                    else:
                        # top-left quadrant + rhs column
                        nc.scalar.copy(
                            ga[:, t, 0:H, 0:H],
                            px[:, 0:H * H].rearrange("p (r c) -> p r c", r=H),
                        )
                        nc.scalar.copy(ga[:, t, :, DP], px[:, H * H:H * H + DP])

            # mirror lower-left quadrant from upper-right (A symmetric)
            ga_sw = ga[:].rearrange("p b r c -> p b c r")
            for b0 in (0, 2):
                nc.scalar.copy(
                    ga[:, b0:b0 + 2, H:DP, 0:H],
                    ga_sw[:, b0:b0 + 2, H:DP, 0:H],
                )

            # ---- add per-test ridge on the diagonal (per 2-block half) ----
            ga_flat = ga[:].rearrange("p b r c -> p b (r c)")
            ga_diag = ga_flat[:, :, ::W + 1]     # [128, NT, 32]
            for b0 in (0, 2):
                nc.vector.tensor_add(
                    ga_diag[:, b0:b0 + 2], ga_diag[:, b0:b0 + 2],
                    regt[:, b0:b0 + 2, None].broadcast_to([P, 2, DP]),
                )

            # ---- batched Gaussian elimination (no pivoting; A is SPD) ----
            # two independent 2-block halves so the scheduler overlaps the
            # first half's elimination with the second half's XWX matmuls
            invp = sb.tile([P, NT, DP], F32)
            fbuf0 = sb.tile([P, 2, D], F32)
            tbuf0 = sb.tile([P, 2, D, DP], F32)
            fbuf1 = sb.tile([P, 2, D], F32)
            tbuf1 = sb.tile([P, 2, D, DP], F32)
            for b0, b1, fbuf, tbuf in ((0, 2, fbuf0, tbuf0),
                                       (2, 4, fbuf1, tbuf1)):
                nb = b1 - b0
                for k in range(DP):
                    nc.vector.reciprocal(
                        invp[:, b0:b1, k], ga[:, b0:b1, k, k])
                    if k == DP - 1:
                        break
                    m = D - k          # rows k+1..31
                    w = DP - k         # cols k+1..32 (incl. rhs)
                    nc.vector.tensor_mul(
                        fbuf[:, :, :m],
                        ga[:, b0:b1, k + 1:DP, k],
                        invp[:, b0:b1, k:k + 1].broadcast_to([P, nb, m]),
                    )
                    nc.vector.tensor_mul(
                        tbuf[:, :, :m, :w],
                        fbuf[:, :, :m, None].broadcast_to([P, nb, m, w]),
                        ga[:, b0:b1, k:k + 1, k + 1:W].broadcast_to(
                            [P, nb, m, w]),
                    )
                    nc.vector.tensor_sub(
                        ga[:, b0:b1, k + 1:DP, k + 1:W],
                        ga[:, b0:b1, k + 1:DP, k + 1:W],
                        tbuf[:, :, :m, :w],
                    )

            # ---- backward elimination on the rhs column (3 ops/step,
            # no reduce): x_k = rhs_k*invp_k; rhs[0:k] -= U[0:k,k]*x_k
            xsol = sb.tile([P, NT, DP], F32)
            bsc = sb.tile([P, NT, D], F32)
            for k in range(DP - 1, -1, -1):
                nc.vector.tensor_mul(
                    xsol[:, :, k], ga[:, :, k, DP], invp[:, :, k]
                )
                if k == 0:
                    break
                nc.vector.tensor_mul(
                    bsc[:, :, :k],
                    ga[:, :, 0:k, k],
                    xsol[:, :, k:k + 1].broadcast_to([P, NT, k]),
                )
                nc.vector.tensor_sub(
                    ga[:, :, 0:k, DP], ga[:, :, 0:k, DP], bsc[:, :, :k]
                )

            # ---- predictions: ypred = xtild_test . beta ----
            xtt = sb.tile([P, NT, DP], F32)
            nc.vector.memset(xtt[:, :, 0:1], 1.0)
            nc.scalar.copy(xtt[:, :, 1:DP], xte)
            yp = sb.tile([P, NT], F32)
            prod = sb.tile([P, NT, DP], F32)
            nc.vector.tensor_mul(prod, xtt, xsol)
            nc.vector.tensor_reduce(
                yp, prod, mybir.AxisListType.X, mybir.AluOpType.add,
            )
            nc.sync.dma_start(
                out_d.rearrange("(t p) one -> p (t one)", p=P), yp
            )

    nc.finalize()
    return nc


_cache: dict[float, object] = {}


def _get_nc(c2: float):
    if c2 not in _cache:
        _cache[c2] = _build_nc(c2)
    return _cache[c2]


def _build_xT(Xtrain, shard):
    """Pack [XtrT | XteT] with chunks at partition offsets 32g for 4-way
    row-group gram matmuls."""
    out = np.zeros((P, 4 * P + TS), np.float32)
    XtrT = Xtrain.T
    for g in range(4):
        for cc in range(4):
            c = 4 * cc + g
            out[32 * g:32 * g + D, cc * P:(cc + 1) * P] = \
                XtrT[:, c * P:(c + 1) * P]
        out[32 * g:32 * g + D, 4 * P:] = shard.T
    return out


def kernel(Ytrain, Xtrain, Xtest, log_lengthscale, _trace=False):
    Ytrain = np.ascontiguousarray(np.asarray(Ytrain, dtype=np.float32))
    Xtrain = np.ascontiguousarray(np.asarray(Xtrain, dtype=np.float32))
    Xtest = np.ascontiguousarray(np.asarray(Xtest, dtype=np.float32))
    lls = float(np.asarray(log_lengthscale, dtype=np.float32))
    c2 = float(np.exp(np.float32(-2.0 * lls)))

    nc = _get_nc(c2)
    in_maps = []
    for core in range(NCORES):
        shard = np.ascontiguousarray(Xtest[core * TS:(core + 1) * TS])
        in_maps.append({
            "xtrain": Xtrain,
            "ytrain": Ytrain,
            "xtest": shard,
            "xT": _build_xT(Xtrain, shard),
        })
    res = run_bass_kernel_spmd(nc, in_maps, list(range(NCORES)),
                               trace=bool(_trace))
    outs = [np.asarray(res.results[c]["ypred"], dtype=np.float32)
            for c in range(NCORES)]
    full = np.concatenate(outs, axis=0)
    if _trace:
        return full, res
    return full



# revision 4
# speedup vs baseline: 2.0115x; 2.0115x over previous
"""Trainium2 Bass kernel: batched locally-weighted ridge regression.

Per test point t: K[t,n] = exp(-|xte_t - xtr_n|^2 / (2 ls^2));
  A_t = Xtild^T diag(K[t]) Xtild + REG*I ; b_t = Xtild^T (K[t] * Y)
  ypred_t = xtild_t . A_t^{-1} b_t
Sharding: data-parallel over the 4096 test points -> 8 cores x 512.

On-device math uses a scaled kernel K'[t,n] = exp((S[n,t] - sn[n]/2) * c2)
(c2 = 1/ls^2); the dropped exp(-st*c2/2) per-test factor rescales A_t and
b_t identically, so beta is preserved by using a per-test ridge
REG_t = REG * exp(st*c2/2).

ypred is computed via a BORDERED elimination: per system the 33x33 matrix
  M = [[A, b], [xtild_t^T, 0]]
is reduced by 32 steps of Gaussian elimination (no pivoting; A is SPD),
after which M[32,32] = -xtild_t^T A^{-1} b = -ypred_t.  This removes the
back-substitution and prediction dot-product phases entirely.

Engine split per core:
  PE  : gram S = XtrT @ XteT (fp32, t-half tiles), XWX/XWy matmuls in
        fp32r (1 cycle/row for free size >= 256)
  ACT : fused exp(S*c2 - sn*c2/2), PSUM evacuations, border-row init,
        output negation
  DVE : Z-build part, norms, reciprocals, left-column elimination updates
  Pool: Z-build part, right-column elimination updates (gpsimd
        tensor_tensor runs at 0.83 ns/elem vs DVE 1.04)
"""

import numpy as np

import concourse.bacc as bacc
import concourse.bass as bass
import concourse.mybir as mybir
from concourse.bass import ds, ts
from concourse.bass_utils import run_bass_kernel_spmd
from concourse.tile import TileContext

F32 = mybir.dt.float32
F32R = mybir.dt.float32r
P = 128
N_TRAIN = 2048
D = 31
DP = 32          # 1 + D
DB = 33          # bordered system size (DP rows of A + border row)
N_TEST = 4096
NCORES = 8
TS = N_TEST // NCORES   # 512 test points per core
NT = TS // P            # 4 t-tiles
NK = N_TRAIN // P       # 16 train chunks
REG = 1e-6
LNREG = float(np.log(REG))
H = 16
NZ = DP * H + H * H + DP             # 512 + 256 + 32 = 800
MM_FP32R = True      # XWX/XWy matmuls in f32r (gram stays fp32)
WLF = 0.44           # fraction of elimination columns updated on DVE


def _build_nc(c2: float):
    """Build the single-core Bass program (SPMD across 8 cores)."""
    nc = bacc.Bacc(trn_type="TRN2")

    xtr_d = nc.dram_tensor("xtrain", [N_TRAIN, D], F32, kind="ExternalInput")
    ytr_d = nc.dram_tensor("ytrain", [N_TRAIN, 1], F32, kind="ExternalInput")
    xte_d = nc.dram_tensor("xtest", [TS, D], F32, kind="ExternalInput")
    # transposed features [XtrT | XteT] on partitions 0..D-1
    xT_d = nc.dram_tensor("xT", [DP, N_TRAIN + TS], F32, kind="ExternalInput")
    out_d = nc.dram_tensor("ypred", [TS, 1], F32, kind="ExternalOutput")

    c2h = 0.5 * c2
    MMDT = F32R if MM_FP32R else F32

    with TileContext(nc) as tc:
        with (
            tc.tile_pool(name="const", bufs=1) as const,
            tc.tile_pool(name="sb", bufs=1) as sb,
            tc.tile_pool(name="pgram", bufs=4, space="PSUM") as pgram,
            tc.tile_pool(name="pxwx", bufs=4, space="PSUM") as pxwx,
        ):
            # ---- load inputs ----
            xtr = sb.tile([P, NK, D], F32)       # natural layout chunks
            nc.sync.dma_start(
                xtr, xtr_d.rearrange("(c p) d -> p c d", p=P)
            )
            ytr = sb.tile([P, NK], F32)
            nc.sync.dma_start(
                ytr, ytr_d.rearrange("(c p) one -> p (c one)", p=P)
            )
            xte = sb.tile([P, NT, D], F32)
            nc.sync.dma_start(
                xte, xte_d.rearrange("(t p) d -> p t d", p=P)
            )
            xT = sb.tile([DP, N_TRAIN + TS], F32)
            nc.sync.dma_start(xT, xT_d[:, :])

            # ---- Xtild chunks [128, NK, 32] (ones column + Xtrain) ----
            xt = sb.tile([P, NK, DP], F32)
            nc.vector.memset(xt[:, :, 0:1], 1.0)
            nc.scalar.copy(xt[:, :, 1:DP], xtr)

            # ---- Z = [xtild_d * xtild_e (768 unique) | xtild * y (32)] ----
            # cols 0:512   : (d, e) for d in 0..31, e in 16..31
            # cols 512:768 : (d, e) for d, e in 0..15
            # cols 768:800 : xtild * y
            zz = sb.tile([P, NK, NZ], MMDT)
            zzv = zz[:, :, 0:DP * H].rearrange("p k (d e) -> p k d e", d=DP)
            # split the 512-col block between Pool (e 16..23) and DVE
            nc.gpsimd.tensor_mul(
                zzv[:, :, :, 0:8],
                xt[:, :, :, None].broadcast_to([P, NK, DP, 8]),
                xt[:, :, None, H:H + 8].broadcast_to([P, NK, DP, 8]),
            )
            nc.vector.tensor_mul(
                zz[:, :, DP * H:DP * H + H * H].rearrange(
                    "p k (d e) -> p k d e", d=H),
                xt[:, :, 0:H, None].broadcast_to([P, NK, H, H]),
                xt[:, :, None, 0:H].broadcast_to([P, NK, H, H]),
            )
            nc.vector.tensor_mul(
                zz[:, :, DP * H + H * H:], xt,
                ytr[:, :, None].broadcast_to([P, NK, DP]),
            )
            nc.vector.tensor_mul(
                zzv[:, :, :, 8:16],
                xt[:, :, :, None].broadcast_to([P, NK, DP, 8]),
                xt[:, :, None, H + 8:DP].broadcast_to([P, NK, DP, 8]),
            )

            # ---- squared norms and per-partition exp biases (Pool) ----
            sqtr = sb.tile([P, NK, D], F32)
            sn = sb.tile([P, NK], F32)
            nc.gpsimd.tensor_mul(sqtr, xtr, xtr)
            nc.vector.tensor_reduce(
                sn, sqtr, mybir.AxisListType.X, mybir.AluOpType.add,
            )
            bias_n = sb.tile([P, NK], F32)       # -sn * c2/2
            nc.gpsimd.tensor_scalar_mul(bias_n, sn, -c2h)
            sqte = sb.tile([P, NT, D], F32)
            st = sb.tile([P, NT], F32)
            nc.gpsimd.tensor_mul(sqte, xte, xte)
            nc.vector.tensor_reduce(
                st, sqte, mybir.AxisListType.X, mybir.AluOpType.add,
            )
            # per-test ridge REG_t = exp(st*c2/2 + ln(REG)), [128, NT]
            lnreg_t = const.tile([P, 1], F32)
            nc.vector.memset(lnreg_t, LNREG)
            regt = sb.tile([P, NT], F32)
            nc.scalar.activation(
                regt, st, mybir.ActivationFunctionType.Exp,
                bias=lnreg_t[:, :], scale=c2h,
            )

            # ---- ga: bordered systems [A | b ; xtild_t^T | 0] ----
            ga = sb.tile([P, NT, DB, DB], F32)
            # border row (row 32): [1, xte_t, 0] — disjoint from evacs
            nc.vector.memset(ga[:, :, DP, 0:1], 1.0)
            nc.vector.memset(ga[:, :, DP, DP:DB], 0.0)
            nc.scalar.copy(ga[:, :, DP, 1:DP], xte)

            # ---- gram (fp32) + exp per t-half; XWX (f32r) per t-tile ----
            kp = sb.tile([P, NK, TS], MMDT)
            ga_sw = ga[:].rearrange("p b r c -> p b c r")
            ga_flat = ga[:].rearrange("p b r c -> p b (r c)")
            ga_diag = ga_flat[:, :, ::DB + 1]    # [128, NT, 33] diagonal
            for h in range(2):
                hc = slice(N_TRAIN + h * 2 * P, N_TRAIN + (h + 1) * 2 * P)
                for c in range(NK):
                    sg = pgram.tile([P, 2 * P], F32, tag="sg")
                    nc.tensor.matmul(
                        sg,
                        xT[0:D, ts(c, P)],
                        xT[0:D, hc],
                        start=True, stop=True,
                    )
                    nc.scalar.activation(
                        kp[:, c, ds(h * 2 * P, 2 * P)], sg,
                        mybir.ActivationFunctionType.Exp,
                        bias=bias_n[:, ds(c, 1)], scale=c2,
                    )
                for t in range(2 * h, 2 * h + 2):
                    # (512:800) group first: its zz cols are ready earliest
                    for (c0, c1) in ((DP * H, NZ), (0, DP * H)):
                        w = c1 - c0
                        px = pxwx.tile([P, 512], F32, tag="px")
                        for c in range(NK):
                            nc.tensor.matmul(
                                px[:, :w],
                                kp[:, c, ts(t, P)],
                                zz[:, c, c0:c1],
                                start=(c == 0), stop=(c == NK - 1),
                            )
                        if c0 == 0:
                            # cols e=16..31, all rows d
                            nc.scalar.copy(
                                ga[:, t, 0:DP, H:DP],
                                px[:, :w].rearrange(
                                    "p (r c) -> p r c", r=DP),
                            )
                        else:
                            # top-left quadrant + rhs column
                            nc.scalar.copy(
                                ga[:, t, 0:H, 0:H],
                                px[:, 0:H * H].rearrange(
                                    "p (r c) -> p r c", r=H),
                            )
                            nc.scalar.copy(
                                ga[:, t, 0:DP, DP], px[:, H * H:H * H + DP])
                    # mirror lower-left quadrant (A symmetric)
                    nc.scalar.copy(
                        ga[:, ds(t, 1), H:DP, 0:H],
                        ga_sw[:, ds(t, 1), H:DP, 0:H],
                    )
                # ridge on the diagonal for this half's two t-tiles
                b0 = 2 * h
                nc.vector.tensor_add(
                    ga_diag[:, b0:b0 + 2, 0:DP],
                    ga_diag[:, b0:b0 + 2, 0:DP],
                    regt[:, b0:b0 + 2, None].broadcast_to([P, 2, DP]),
                )

            # ---- bordered Gaussian elimination, 2 chains of 2 t-tiles ----
            # Per step k: eliminate col k from rows k+1..32 (incl. border
            # row).  DVE updates the left wL columns, Pool the rest.
            invp = sb.tile([P, NT, DP], F32)
            fbuf = sb.tile([P, 2, DP], F32)
            tbL = sb.tile([P, 2, DP, DP], F32)
            tbR = sb.tile([P, 2, DP, DP], F32)
            yp = sb.tile([P, NT], F32)
            outv = out_d.rearrange("(t p) one -> p (t one)", p=P)
            for b0 in (0, 2):
                bs = slice(b0, b0 + 2)
                for k in range(DP):
                    m = DP - k       # rows k+1..32
                    w = DP - k       # cols k+1..32
                    nc.vector.reciprocal(
                        invp[:, bs, k], ga[:, bs, k, k])
                    fb = fbuf[:, :, :m]
                    nc.vector.tensor_mul(
                        fb,
                        ga[:, bs, k + 1:DB, k],
                        invp[:, bs, k:k + 1].broadcast_to([P, 2, m]),
                    )
                    wL = w if w <= 2 else max(1, int(round(WLF * w)))
                    wR = w - wL
                    nc.vector.tensor_mul(
                        tbL[:, :, :m, :wL],
                        fb[:, :, :, None].broadcast_to([P, 2, m, wL]),
                        ga[:, bs, k:k + 1, k + 1:k + 1 + wL].broadcast_to(
                            [P, 2, m, wL]),
                    )
                    nc.vector.tensor_sub(
                        ga[:, bs, k + 1:DB, k + 1:k + 1 + wL],
                        ga[:, bs, k + 1:DB, k + 1:k + 1 + wL],
                        tbL[:, :, :m, :wL],
                    )
                    if wR:
                        nc.gpsimd.tensor_mul(
                            tbR[:, :, :m, :wR],
                            fb[:, :, :, None].broadcast_to([P, 2, m, wR]),
                            ga[:, bs, k:k + 1, k + 1 + wL:DB].broadcast_to(
                                [P, 2, m, wR]),
                        )
                        nc.gpsimd.tensor_sub(
                            ga[:, bs, k + 1:DB, k + 1 + wL:DB],
                            ga[:, bs, k + 1:DB, k + 1 + wL:DB],
                            tbR[:, :, :m, :wR],
                        )
                # ypred = -M[32, 32] for this chain's two t-tiles
                nc.scalar.mul(yp[:, bs], ga[:, bs, DP, DP], -1.0)
                nc.sync.dma_start(outv[:, bs], yp[:, bs])

    nc.finalize()
    return nc


_cache: dict[float, object] = {}


def _get_nc(c2: float):
    if c2 not in _cache:
        _cache[c2] = _build_nc(c2)
    return _cache[c2]


def _build_xT(Xtrain, shard):
    """Pack [XtrT | XteT] on partitions 0..D-1."""
    out = np.zeros((DP, N_TRAIN + TS), np.float32)
    out[0:D, 0:N_TRAIN] = Xtrain.T
    out[0:D, N_TRAIN:] = shard.T
    return out


def kernel(Ytrain, Xtrain, Xtest, log_lengthscale, _trace=False):
    Ytrain = np.ascontiguousarray(np.asarray(Ytrain, dtype=np.float32))
    Xtrain = np.ascontiguousarray(np.asarray(Xtrain, dtype=np.float32))
    Xtest = np.ascontiguousarray(np.asarray(Xtest, dtype=np.float32))
    lls = float(np.asarray(log_lengthscale, dtype=np.float32))
    c2 = float(np.exp(np.float32(-2.0 * lls)))

    nc = _get_nc(c2)
    in_maps = []
    for core in range(NCORES):
        shard = np.ascontiguousarray(Xtest[core * TS:(core + 1) * TS])
        in_maps.append({
            "xtrain": Xtrain,
            "ytrain": Ytrain,
            "xtest": shard,
            "xT": _build_xT(Xtrain, shard),
        })
    res = run_bass_kernel_spmd(nc, in_maps, list(range(NCORES)),
                               trace=bool(_trace))
    outs = [np.asarray(res.results[c]["ypred"], dtype=np.float32)
            for c in range(NCORES)]
    full = np.concatenate(outs, axis=0)
    if _trace:
        return full, res
    return full


# revision 6
# speedup vs baseline: 2.0323x; 1.0103x over previous
"""Trainium2 Bass kernel: batched locally-weighted ridge regression.

Per test point t: K[t,n] = exp(-|xte_t - xtr_n|^2 / (2 ls^2));
  A_t = Xtild^T diag(K[t]) Xtild + REG*I ; b_t = Xtild^T (K[t] * Y)
  ypred_t = xtild_t . A_t^{-1} b_t
Sharding: data-parallel over the 4096 test points -> 8 cores x 512.

On-device math uses a scaled kernel K'[t,n] = exp((S[n,t] - sn[n]/2) * c2)
(c2 = 1/ls^2); the dropped exp(-st*c2/2) per-test factor rescales A_t and
b_t identically, so beta is preserved by using a per-test ridge
REG_t = REG * exp(st*c2/2).

ypred is computed via a BORDERED elimination: per system the 33x33 matrix
  M = [[A, b], [xtild_t^T, 0]]
is reduced by 32 steps of Gaussian elimination (no pivoting; A is SPD),
after which M[32,32] = -xtild_t^T A^{-1} b = -ypred_t.  This removes the
back-substitution and prediction dot-product phases entirely.

Engine split per core:
  PE  : gram S = XtrT @ XteT and XWX/XWy matmuls, all in fp32r
        (1 cycle/row at free size >= 256)
  ACT : fused exp(S*c2 - sn*c2/2), PSUM evacuations, border-row init,
        bias_n, output negation
  DVE : Z-build part, norms reduces, left-column elimination updates
        (multiplier column via divide, no reciprocal step)
  Pool: Z-build part, squared norms, right-column elimination updates
        (gpsimd tensor_tensor runs at 0.83 ns/elem vs DVE 1.04)

The elimination runs as four single-t-tile chains so chain b starts as
soon as XWX tile b is evacuated; each chain keeps a parity-double-
buffered multiplier column so the Pool lags DVE freely without
write-after-read stalls.
"""

import numpy as np

import concourse.bacc as bacc
import concourse.bass as bass
import concourse.mybir as mybir
from concourse.bass import ds, ts
from concourse.bass_utils import run_bass_kernel_spmd
from concourse.tile import TileContext

F32 = mybir.dt.float32
F32R = mybir.dt.float32r
P = 128
N_TRAIN = 2048
D = 31
DP = 32          # 1 + D
DB = 33          # bordered system size (DP rows of A + border row)
N_TEST = 4096
NCORES = 8
TS = N_TEST // NCORES   # 512 test points per core
NT = TS // P            # 4 t-tiles
NK = N_TRAIN // P       # 16 train chunks
REG = 1e-6
LNREG = float(np.log(REG))
H = 16
NZ = DP * H + H * H + DP             # 512 + 256 + 32 = 800
WLF = 0.43           # fraction of elimination columns updated on DVE


def _build_nc(c2: float):
    """Build the single-core Bass program (SPMD across 8 cores)."""
    nc = bacc.Bacc(trn_type="TRN2")

    xtr_d = nc.dram_tensor("xtrain", [N_TRAIN, D], F32, kind="ExternalInput")
    ytr_d = nc.dram_tensor("ytrain", [N_TRAIN, 1], F32, kind="ExternalInput")
    xte_d = nc.dram_tensor("xtest", [TS, D], F32, kind="ExternalInput")
    # transposed features [XtrT | XteT] on partitions 0..D-1
    xT_d = nc.dram_tensor("xT", [DP, N_TRAIN + TS], F32, kind="ExternalInput")
    out_d = nc.dram_tensor("ypred", [TS, 1], F32, kind="ExternalOutput")

    c2h = 0.5 * c2

    with TileContext(nc) as tc:
        with (
            tc.tile_pool(name="const", bufs=1) as const,
            tc.tile_pool(name="sb", bufs=1) as sb,
            tc.tile_pool(name="pgram", bufs=4, space="PSUM") as pgram,
            tc.tile_pool(name="pxwx", bufs=4, space="PSUM") as pxwx,
        ):
            # ---- load inputs (xT first and split: it gates the grams) ----
            xT = sb.tile([DP, N_TRAIN + TS], F32)
            nc.sync.dma_start(xT[:, 0:N_TRAIN], xT_d[:, 0:N_TRAIN])
            nc.sync.dma_start(xT[:, N_TRAIN:], xT_d[:, N_TRAIN:])
            xtr = sb.tile([P, NK, D], F32)       # natural layout chunks
            nc.sync.dma_start(
                xtr, xtr_d.rearrange("(c p) d -> p c d", p=P)
            )
            ytr = sb.tile([P, NK], F32)
            nc.sync.dma_start(
                ytr, ytr_d.rearrange("(c p) one -> p (c one)", p=P)
            )
            xte = sb.tile([P, NT, D], F32)
            nc.sync.dma_start(
                xte, xte_d.rearrange("(t p) d -> p t d", p=P)
            )

            # ---- Xtild chunks [128, NK, 32] (ones column + Xtrain) ----
            xt = sb.tile([P, NK, DP], F32)
            nc.vector.memset(xt[:, :, 0:1], 1.0)
            nc.scalar.copy(xt[:, :, 1:DP], xtr)

            # ---- squared train norms -> exp bias (feeds the exps early) --
            sqtr = sb.tile([P, NK, D], F32)
            sn = sb.tile([P, NK], F32)
            nc.gpsimd.tensor_mul(sqtr, xtr, xtr)
            nc.vector.tensor_reduce(
                sn, sqtr, mybir.AxisListType.X, mybir.AluOpType.add,
            )
            bias_n = sb.tile([P, NK], F32)       # -sn * c2/2
            nc.scalar.mul(bias_n, sn, -c2h)

            # ---- Z = [xtild_d * xtild_e (768 unique) | xtild * y (32)] ----
            # cols 0:512   : (d, e) for d in 0..31, e in 16..31
            # cols 512:768 : (d, e) for d, e in 0..15
            # cols 768:800 : xtild * y
            zz = sb.tile([P, NK, NZ], F32R)
            zzv = zz[:, :, 0:DP * H].rearrange("p k (d e) -> p k d e", d=DP)
            # DVE builds what the (512:800) matmul group needs
            nc.vector.tensor_mul(
                zz[:, :, DP * H:DP * H + H * H].rearrange(
                    "p k (d e) -> p k d e", d=H),
                xt[:, :, 0:H, None].broadcast_to([P, NK, H, H]),
                xt[:, :, None, 0:H].broadcast_to([P, NK, H, H]),
            )
            nc.vector.tensor_mul(
                zz[:, :, DP * H + H * H:], xt,
                ytr[:, :, None].broadcast_to([P, NK, DP]),
            )
            # Pool builds the (0:512) group columns
            nc.gpsimd.tensor_mul(
                zzv[:, :, :, 0:8],
                xt[:, :, :, None].broadcast_to([P, NK, DP, 8]),
                xt[:, :, None, H:H + 8].broadcast_to([P, NK, DP, 8]),
            )
            nc.gpsimd.tensor_mul(
                zzv[:, :, :, 8:16],
                xt[:, :, :, None].broadcast_to([P, NK, DP, 8]),
                xt[:, :, None, H + 8:DP].broadcast_to([P, NK, DP, 8]),
            )

            # ---- test norms -> per-test ridge ----
            sqte = sb.tile([P, NT, D], F32)
            st = sb.tile([P, NT], F32)
            nc.gpsimd.tensor_mul(sqte, xte, xte)
            nc.vector.tensor_reduce(
                st, sqte, mybir.AxisListType.X, mybir.AluOpType.add,
            )
            # REG_t = exp(st*c2/2 + ln(REG)), [128, NT]
            lnreg_t = const.tile([P, 1], F32)
            nc.vector.memset(lnreg_t, LNREG)
            regt = sb.tile([P, NT], F32)
            nc.scalar.activation(
                regt, st, mybir.ActivationFunctionType.Exp,
                bias=lnreg_t[:, :], scale=c2h,
            )

            # ---- ga: bordered systems [A | b ; xtild_t^T | 0] ----
            ga = sb.tile([P, NT, DB, DB], F32)
            # border row (row 32): [1, xte_t, 0] — disjoint from evacs
            nc.vector.memset(ga[:, :, DP, 0:1], 1.0)
            nc.vector.memset(ga[:, :, DP, DP:DB], 0.0)
            nc.scalar.copy(ga[:, :, DP, 1:DP], xte)

            # ---- gram + exp per t-half; XWX per t-tile; eliminate ----
            kp = sb.tile([P, NK, TS], F32R)
            ga_sw = ga[:].rearrange("p b r c -> p b c r")
            ga_flat = ga[:].rearrange("p b r c -> p b (r c)")
            ga_diag = ga_flat[:, :, ::DB + 1]    # [128, NT, 33] diagonal
            xTr = xT[:, :].bitcast(F32R)
            fbufs = [sb.tile([P, 1, 2, DP], F32, name=f"fbuf{i}")
                     for i in range(NT)]
            tbL = sb.tile([P, DP, DP], F32)
            tbR = sb.tile([P, DP, DP], F32)
            yp = sb.tile([P, NT], F32)
            outv = out_d.rearrange("(t p) one -> p (t one)", p=P)

            def eliminate(b):
                """Bordered elimination chain for t-tile b."""
                fbuf = fbufs[b]
                bs = ds(b, 1)
                for k in range(DP):
                    m = DP - k       # rows k+1..32
                    w = DP - k       # cols k+1..32
                    fb = fbuf[:, :, k % 2, :m]
                    nc.vector.tensor_tensor(
                        fb,
                        ga[:, bs, k + 1:DB, k],
                        ga[:, bs, k, k:k + 1].broadcast_to([P, 1, m]),
                        mybir.AluOpType.divide,
                    )
                    wL = w if w <= 2 else max(1, int(round(WLF * w)))
                    wR = w - wL
                    nc.vector.tensor_mul(
                        tbL[:, :m, :wL],
                        fb[:, 0, :, None].broadcast_to([P, m, wL]),
                        ga[:, b, k:k + 1, k + 1:k + 1 + wL].broadcast_to(
                            [P, m, wL]),
                    )
                    nc.vector.tensor_sub(
                        ga[:, b, k + 1:DB, k + 1:k + 1 + wL],
                        ga[:, b, k + 1:DB, k + 1:k + 1 + wL],
                        tbL[:, :m, :wL],
                    )
                    if wR:
                        nc.gpsimd.tensor_mul(
                            tbR[:, :m, :wR],
                            fb[:, 0, :, None].broadcast_to([P, m, wR]),
                            ga[:, b, k:k + 1, k + 1 + wL:DB].broadcast_to(
                                [P, m, wR]),
                        )
                        nc.gpsimd.tensor_sub(
                            ga[:, b, k + 1:DB, k + 1 + wL:DB],
                            ga[:, b, k + 1:DB, k + 1 + wL:DB],
                            tbR[:, :m, :wR],
                        )
                # ypred = -M[32, 32]
                nc.scalar.mul(yp[:, bs], ga[:, bs, DP, DP], -1.0)

            for h in range(2):
                hc = slice(N_TRAIN + h * 2 * P, N_TRAIN + (h + 1) * 2 * P)
                for c in range(NK):
                    sg = pgram.tile([P, 2 * P], F32, tag="sg")
                    nc.tensor.matmul(
                        sg,
                        xTr[0:D, ts(c, P)],
                        xTr[0:D, hc],
                        start=True, stop=True,
                    )
                    nc.scalar.activation(
                        kp[:, c, ds(h * 2 * P, 2 * P)], sg,
                        mybir.ActivationFunctionType.Exp,
                        bias=bias_n[:, ds(c, 1)], scale=c2,
                    )
                for t in range(2 * h, 2 * h + 2):
                    # (512:800) group first: its zz cols are ready earliest
                    for (c0, c1) in ((DP * H, NZ), (0, DP * H)):
                        w = c1 - c0
                        px = pxwx.tile([P, 512], F32, tag="px")
                        for c in range(NK):
                            nc.tensor.matmul(
                                px[:, :w],
                                kp[:, c, ts(t, P)],
                                zz[:, c, c0:c1],
                                start=(c == 0), stop=(c == NK - 1),
                            )
                        if c0 == 0:
                            # cols e=16..31, all rows d
                            nc.scalar.copy(
                                ga[:, t, 0:DP, H:DP],
                                px[:, :w].rearrange(
                                    "p (r c) -> p r c", r=DP),
                            )
                        else:
                            # top-left quadrant + rhs column
                            nc.scalar.copy(
                                ga[:, t, 0:H, 0:H],
                                px[:, 0:H * H].rearrange(
                                    "p (r c) -> p r c", r=H),
                            )
                            nc.scalar.copy(
                                ga[:, t, 0:DP, DP], px[:, H * H:H * H + DP])
                    # mirror lower-left quadrant (A symmetric)
                    nc.scalar.copy(
                        ga[:, ds(t, 1), H:DP, 0:H],
                        ga_sw[:, ds(t, 1), H:DP, 0:H],
                    )
                    # ridge on the diagonal, then eliminate this t-tile
                    nc.vector.tensor_add(
                        ga_diag[:, ds(t, 1), 0:DP],
                        ga_diag[:, ds(t, 1), 0:DP],
                        regt[:, t:t + 1, None].broadcast_to([P, 1, DP]),
                    )
                    eliminate(t)

            nc.sync.dma_start(outv, yp)

    nc.finalize()
    return nc


_cache: dict[float, object] = {}


def _get_nc(c2: float):
    if c2 not in _cache:
        _cache[c2] = _build_nc(c2)
    return _cache[c2]


def _build_xT(Xtrain, shard):
    """Pack [XtrT | XteT] on partitions 0..D-1."""
    out = np.zeros((DP, N_TRAIN + TS), np.float32)
    out[0:D, 0:N_TRAIN] = Xtrain.T
    out[0:D, N_TRAIN:] = shard.T
    return out


def kernel(Ytrain, Xtrain, Xtest, log_lengthscale, _trace=False):
    Ytrain = np.ascontiguousarray(np.asarray(Ytrain, dtype=np.float32))
    Xtrain = np.ascontiguousarray(np.asarray(Xtrain, dtype=np.float32))
    Xtest = np.ascontiguousarray(np.asarray(Xtest, dtype=np.float32))
    lls = float(np.asarray(log_lengthscale, dtype=np.float32))
    c2 = float(np.exp(np.float32(-2.0 * lls)))

    nc = _get_nc(c2)
    in_maps = []
    for core in range(NCORES):
        shard = np.ascontiguousarray(Xtest[core * TS:(core + 1) * TS])
        in_maps.append({
            "xtrain": Xtrain,
            "ytrain": Ytrain,
            "xtest": shard,
            "xT": _build_xT(Xtrain, shard),
        })
    res = run_bass_kernel_spmd(nc, in_maps, list(range(NCORES)),
                               trace=bool(_trace))
    outs = [np.asarray(res.results[c]["ypred"], dtype=np.float32)
            for c in range(NCORES)]
    full = np.concatenate(outs, axis=0)
    if _trace:
        return full, res
    return full


# revision 7
# speedup vs baseline: 2.3586x; 1.1606x over previous
"""Trainium2 Bass kernel: batched locally-weighted ridge regression.

Per test point t: K[t,n] = exp(-|xte_t - xtr_n|^2 / (2 ls^2));
  A_t = Xtild^T diag(K[t]) Xtild + REG*I ; b_t = Xtild^T (K[t] * Y)
  ypred_t = xtild_t . A_t^{-1} b_t
Sharding: data-parallel over the 4096 test points -> 8 cores x 512.

On-device math uses a scaled kernel K'[t,n] = exp((S[n,t] - sn[n]/2) * c2)
(c2 = 1/ls^2); the dropped exp(-st*c2/2) per-test factor rescales A_t and
b_t identically, so beta is preserved by using a per-test ridge
REG_t = REG * exp(st*c2/2).

ypred is computed via a BORDERED elimination: per system the 33x33 matrix
  M = [[A, b], [xtild_t^T, 0]]
is reduced by 32 steps of Gaussian elimination (no pivoting; A is SPD),
after which M[32,32] = -xtild_t^T A^{-1} b = -ypred_t.  This removes the
back-substitution and prediction dot-product phases entirely.

Engine split per core:
  PE  : gram S = XtrT @ XteT and XWX/XWy matmuls, all in fp32r
        (1 cycle/row at free size >= 256)
  ACT : fused exp(S*c2 - sn*c2/2), PSUM evacuations, border-row init,
        bias_n, output negation
  DVE : Z-build part, norm reduces, left-column elimination updates
        (multiplier column via divide, no reciprocal step)
  Pool: Z-build part, squared norms, right-column elimination updates
        (gpsimd tensor_tensor runs at 0.83 ns/elem vs DVE 1.04)

The elimination runs as two 2-t-tile chains; each keeps a parity-double-
buffered multiplier column so Pool lags DVE freely without
write-after-read stalls.  xtrain/xtest/ytrain are host-packed into one
DMA to cut the serial SP dma-issue cost at startup.
"""

import numpy as np

import concourse.bacc as bacc
import concourse.bass as bass
import concourse.mybir as mybir
from concourse.bass import ds, ts
from concourse.bass_utils import run_bass_kernel_spmd
from concourse.tile import TileContext

F32 = mybir.dt.float32
F32R = mybir.dt.float32r
P = 128
N_TRAIN = 2048
D = 31
DP = 32          # 1 + D
DB = 33          # bordered system size (DP rows of A + border row)
N_TEST = 4096
NCORES = 8
TS = N_TEST // NCORES   # 512 test points per core
NT = TS // P            # 4 t-tiles
NK = N_TRAIN // P       # 16 train chunks
REG = 1e-6
LNREG = float(np.log(REG))
H = 16
NZ = DP * H + H * H + DP             # 512 + 256 + 32 = 800
WLF = 0.37           # fraction of elimination columns updated on DVE
NMISC = NK * D + NT * D + NK         # host-packed xtr | xte | ytr cols


def _build_nc(c2: float):
    """Build the single-core Bass program (SPMD across 8 cores)."""
    nc = bacc.Bacc(trn_type="TRN2")

    xm_d = nc.dram_tensor("xmisc", [P, NMISC], F32, kind="ExternalInput")
    xT_d = nc.dram_tensor("xT", [DP, N_TRAIN + TS], F32, kind="ExternalInput")
    out_d = nc.dram_tensor("ypred", [TS, 1], F32, kind="ExternalOutput")

    c2h = 0.5 * c2

    with TileContext(nc) as tc:
        with (
            tc.tile_pool(name="const", bufs=1) as const,
            tc.tile_pool(name="sb", bufs=1) as sb,
            tc.tile_pool(name="pgram", bufs=4, space="PSUM") as pgram,
            tc.tile_pool(name="pxwx", bufs=4, space="PSUM") as pxwx,
        ):
            # ---- load inputs: 2 DMAs (packed misc, transposed feats) ----
            xm = sb.tile([P, NMISC], F32)
            nc.sync.dma_start(xm, xm_d[:, :])
            xT = sb.tile([DP, N_TRAIN + TS], F32)
            nc.sync.dma_start(xT, xT_d[:, :])
            xtr = xm[:, 0:NK * D].rearrange("p (c d) -> p c d", c=NK)
            xte = xm[:, NK * D:NK * D + NT * D].rearrange(
                "p (t d) -> p t d", t=NT)
            ytr = xm[:, NK * D + NT * D:]

            # ---- squared train norms -> exp bias (feeds the exps) ----
            sqtr = sb.tile([P, NK, D], F32)
            sn = sb.tile([P, NK], F32)
            nc.gpsimd.tensor_mul(sqtr, xtr, xtr)
            nc.vector.tensor_reduce(
                sn, sqtr, mybir.AxisListType.X, mybir.AluOpType.add,
            )
            bias_n = sb.tile([P, NK], F32)       # -sn * c2/2
            nc.scalar.mul(bias_n, sn, -c2h)

            # ---- Xtild chunks [128, NK, 32] (ones column + Xtrain) ----
            xt = sb.tile([P, NK, DP], F32)
            nc.vector.memset(xt[:, :, 0:1], 1.0)
            nc.scalar.copy(xt[:, :, 1:DP], xtr)

            # ---- Z = [xtild_d * xtild_e (768 unique) | xtild * y (32)] ----
            # cols 0:512   : (d, e) for d in 0..31, e in 16..31   (Pool)
            # cols 512:768 : (d, e) for d, e in 0..15             (DVE)
            # cols 768:800 : xtild * y                            (DVE)
            zz = sb.tile([P, NK, NZ], F32R)
            zzv = zz[:, :, 0:DP * H].rearrange("p k (d e) -> p k d e", d=DP)
            nc.vector.tensor_mul(
                zz[:, :, DP * H:DP * H + H * H].rearrange(
                    "p k (d e) -> p k d e", d=H),
                xt[:, :, 0:H, None].broadcast_to([P, NK, H, H]),
                xt[:, :, None, 0:H].broadcast_to([P, NK, H, H]),
            )
            nc.vector.tensor_mul(
                zz[:, :, DP * H + H * H:], xt,
                ytr[:, :, None].broadcast_to([P, NK, DP]),
            )
            nc.gpsimd.tensor_mul(
                zzv[:, :, :, 0:8],
                xt[:, :, :, None].broadcast_to([P, NK, DP, 8]),
                xt[:, :, None, H:H + 8].broadcast_to([P, NK, DP, 8]),
            )
            nc.gpsimd.tensor_mul(
                zzv[:, :, :, 8:16],
                xt[:, :, :, None].broadcast_to([P, NK, DP, 8]),
                xt[:, :, None, H + 8:DP].broadcast_to([P, NK, DP, 8]),
            )

            # ---- test norms -> per-test ridge ----
            sqte = sb.tile([P, NT, D], F32)
            st = sb.tile([P, NT], F32)
            nc.gpsimd.tensor_mul(sqte, xte, xte)
            nc.vector.tensor_reduce(
                st, sqte, mybir.AxisListType.X, mybir.AluOpType.add,
            )
            # REG_t = exp(st*c2/2 + ln(REG)), [128, NT]
            lnreg_t = const.tile([P, 1], F32)
            nc.vector.memset(lnreg_t, LNREG)
            regt = sb.tile([P, NT], F32)
            nc.scalar.activation(
                regt, st, mybir.ActivationFunctionType.Exp,
                bias=lnreg_t[:, :], scale=c2h,
            )

            # ---- ga: bordered systems [A | b ; xtild_t^T | 0] ----
            ga = sb.tile([P, NT, DB, DB], F32)
            # border row (row 32): [1, xte_t, 0] — disjoint from evacs
            nc.vector.memset(ga[:, :, DP, 0:1], 1.0)
            nc.vector.memset(ga[:, :, DP, DP:DB], 0.0)
            nc.scalar.copy(ga[:, :, DP, 1:DP], xte)

            # ---- gram + exp per t-half; XWX per t-tile; eliminate ----
            kp = sb.tile([P, NK, TS], F32R)
            ga_sw = ga[:].rearrange("p b r c -> p b c r")
            ga_flat = ga[:].rearrange("p b r c -> p b (r c)")
            ga_diag = ga_flat[:, :, ::DB + 1]    # [128, NT, 33] diagonal
            xTr = xT[:, :].bitcast(F32R)
            fbuf0 = sb.tile([P, 2, 2, DP], F32)
            fbuf1 = sb.tile([P, 2, 2, DP], F32)
            tbL = sb.tile([P, 2, DP, DP], F32)
            tbR = sb.tile([P, 2, DP, DP], F32)
            yp = sb.tile([P, NT], F32)
            outv = out_d.rearrange("(t p) one -> p (t one)", p=P)

            def eliminate(b0, fbuf):
                """Bordered elimination chain for t-tiles b0, b0+1."""
                bs = slice(b0, b0 + 2)
                for k in range(DP):
                    m = DP - k       # rows k+1..32
                    w = DP - k       # cols k+1..32
                    fb = fbuf[:, :, k % 2, :m]
                    nc.vector.tensor_tensor(
                        fb,
                        ga[:, bs, k + 1:DB, k],
                        ga[:, bs, k, k:k + 1].broadcast_to([P, 2, m]),
                        mybir.AluOpType.divide,
                    )
                    wL = w if w <= 2 else max(1, int(round(WLF * w)))
                    wR = w - wL
                    nc.vector.tensor_mul(
                        tbL[:, :, :m, :wL],
                        fb[:, :, :, None].broadcast_to([P, 2, m, wL]),
                        ga[:, bs, k:k + 1, k + 1:k + 1 + wL].broadcast_to(
                            [P, 2, m, wL]),
                    )
                    nc.vector.tensor_sub(
                        ga[:, bs, k + 1:DB, k + 1:k + 1 + wL],
                        ga[:, bs, k + 1:DB, k + 1:k + 1 + wL],
                        tbL[:, :, :m, :wL],
                    )
                    if wR:
                        nc.gpsimd.tensor_mul(
                            tbR[:, :, :m, :wR],
                            fb[:, :, :, None].broadcast_to([P, 2, m, wR]),
                            ga[:, bs, k:k + 1, k + 1 + wL:DB].broadcast_to(
                                [P, 2, m, wR]),
                        )
                        nc.gpsimd.tensor_sub(
                            ga[:, bs, k + 1:DB, k + 1 + wL:DB],
                            ga[:, bs, k + 1:DB, k + 1 + wL:DB],
                            tbR[:, :, :m, :wR],
                        )
                # ypred = -M[32, 32] for the chain's two t-tiles
                nc.scalar.mul(yp[:, bs], ga[:, bs, DP, DP], -1.0)
                nc.sync.dma_start(outv[:, bs], yp[:, bs])

            for h in range(2):
                hc = slice(N_TRAIN + h * 2 * P, N_TRAIN + (h + 1) * 2 * P)
                for c in range(NK):
                    sg = pgram.tile([P, 2 * P], F32, tag="sg")
                    nc.tensor.matmul(
                        sg,
                        xTr[0:D, ts(c, P)],
                        xTr[0:D, hc],
                        start=True, stop=True,
                    )
                    nc.scalar.activation(
                        kp[:, c, ds(h * 2 * P, 2 * P)], sg,
                        mybir.ActivationFunctionType.Exp,
                        bias=bias_n[:, ds(c, 1)], scale=c2,
                    )
                for t in range(2 * h, 2 * h + 2):
                    # (512:800) group first: its zz cols are ready earliest
                    for (c0, c1) in ((DP * H, NZ), (0, DP * H)):
                        w = c1 - c0
                        px = pxwx.tile([P, 512], F32, tag="px")
                        for c in range(NK):
                            nc.tensor.matmul(
                                px[:, :w],
                                kp[:, c, ts(t, P)],
                                zz[:, c, c0:c1],
                                start=(c == 0), stop=(c == NK - 1),
                            )
                        if c0 == 0:
                            # cols e=16..31, all rows d
                            nc.scalar.copy(
                                ga[:, t, 0:DP, H:DP],
                                px[:, :w].rearrange(
                                    "p (r c) -> p r c", r=DP),
                            )
                        else:
                            # top-left quadrant + rhs column
                            nc.scalar.copy(
                                ga[:, t, 0:H, 0:H],
                                px[:, 0:H * H].rearrange(
                                    "p (r c) -> p r c", r=H),
                            )
                            nc.scalar.copy(
                                ga[:, t, 0:DP, DP], px[:, H * H:H * H + DP])
                    # mirror lower-left quadrant (A symmetric)
                    nc.scalar.copy(
                        ga[:, ds(t, 1), H:DP, 0:H],
                        ga_sw[:, ds(t, 1), H:DP, 0:H],
                    )
                # ridge on the diagonal, then eliminate this half
                b0 = 2 * h
                nc.vector.tensor_add(
                    ga_diag[:, b0:b0 + 2, 0:DP],
                    ga_diag[:, b0:b0 + 2, 0:DP],
                    regt[:, b0:b0 + 2, None].broadcast_to([P, 2, DP]),
                )
                eliminate(b0, fbuf0 if h == 0 else fbuf1)

    nc.finalize()
    return nc


_cache: dict[float, object] = {}


def _get_nc(c2: float):
    if c2 not in _cache:
        _cache[c2] = _build_nc(c2)
    return _cache[c2]


def _build_xT(Xtrain, shard):
    """Pack [XtrT | XteT] on partitions 0..D-1."""
    out = np.zeros((DP, N_TRAIN + TS), np.float32)
    out[0:D, 0:N_TRAIN] = Xtrain.T
    out[0:D, N_TRAIN:] = shard.T
    return out


def _build_xmisc(Xtrain, shard, Ytrain):
    """Pack xtrain chunks | xtest tiles | ytrain chunks as [128, NMISC]."""
    out = np.empty((P, NMISC), np.float32)
    for c in range(NK):
        out[:, c * D:(c + 1) * D] = Xtrain[c * P:(c + 1) * P]
    o = NK * D
    for t in range(NT):
        out[:, o + t * D:o + (t + 1) * D] = shard[t * P:(t + 1) * P]
    o += NT * D
    for c in range(NK):
        out[:, o + c] = Ytrain[c * P:(c + 1) * P, 0]
    return out


def kernel(Ytrain, Xtrain, Xtest, log_lengthscale, _trace=False):
    Ytrain = np.ascontiguousarray(np.asarray(Ytrain, dtype=np.float32))
    Xtrain = np.ascontiguousarray(np.asarray(Xtrain, dtype=np.float32))
    Xtest = np.ascontiguousarray(np.asarray(Xtest, dtype=np.float32))
    lls = float(np.asarray(log_lengthscale, dtype=np.float32))
    c2 = float(np.exp(np.float32(-2.0 * lls)))

    nc = _get_nc(c2)
    in_maps = []
    for core in range(NCORES):
        shard = np.ascontiguousarray(Xtest[core * TS:(core + 1) * TS])
        in_maps.append({
            "xmisc": _build_xmisc(Xtrain, shard, Ytrain),
            "xT": _build_xT(Xtrain, shard),
        })
    res = run_bass_kernel_spmd(nc, in_maps, list(range(NCORES)),
                               trace=bool(_trace))
    outs = [np.asarray(res.results[c]["ypred"], dtype=np.float32)
            for c in range(NCORES)]
    full = np.concatenate(outs, axis=0)
    if _trace:
        return full, res
    return full
